# revision 4
# baseline (speedup 1.0000x reference)
"""Trainium2 Bass kernel (v9) for LocalDynamicGraph edge-feature construction.

Per batch element b (one NeuronCore each, data-parallel over B=8):
    out[b, n, c, k] = x[b, idx[b,n,k], c] - x[b, n, c]   for c < 64
    out[b, n, c, k] = x[b, n, c - 64]                    for c >= 64

v9 design — fully on-chip gather (no SWDGE, no gather DMA descriptors):
  The v8 SWDGE dma_gather was doubly bottlenecked: Pool-engine descriptor
  generation ran at ~2.28 ns/index (299 us busy for 131072 indices), and
  the 131072 x 128B gather packets kept all 16 SDMA engines ~280 us busy,
  crowding out the 64 MiB of mandatory output writes.  v9 instead:

  - Hosts x transposed in SBUF (xtr: [128, 8192] fp32; channels 0..63
    duplicated on partitions 64..127) and gathers with the GPSIMD
    ap_gather library instruction (SBUF -> SBUF, d=1).  Each of the 8
    GPSIMD cores gathers with the index list stored in its own
    16-partition band, so the two partition halves gather *different*
    point-blocks concurrently: per call, partitions 0-63 fetch block
    pair-halves A, partitions 64-127 fetch halves B.
  - Index order per block is l = k*128 + p, so G[:, q*2048 + k*128 + p]
    holds x[idx[p,k], ch] of the two blocks; each [128, 128] k-slice is
    PE-transposed (fp32, via identity matmul) into PSUM as [p, h*64+c].
  - DVE subtracts the block centers (4 k-slices per op, strided 4D APs)
    straight from PSUM into the (c,k)-interleaved first output half;
    ACT broadcast-copies the centers into the second half.
  - Each finished pair tile [128, 4096] is written back as one 2 MB DMA
    (256 x 8 KB descriptors), so the SDMA engines only ever see large
    sequential traffic: ~7 MB in, 64 MB out per core.
"""

import numpy as np

import concourse.bacc as bacc
import concourse.mybir as mybir
from concourse.masks import make_identity
from concourse.tile import TileContext
from concourse.bass_utils import run_bass_kernel_spmd

# Problem constants (hardcoded per contest contract).
B = 8
N = 8192
C = 64
K = 16
P = 128              # partitions / points per block
NBLK = N // P        # 64 point-blocks per core
GB = 2               # block pairs per ap_gather call (per-call num_idxs = GB*2048)
NI = NBLK // (2 * GB)    # gather calls per core
NIDX = GB * K * P        # indices gathered per 16-partition band per call
CPI = NIDX // 16         # idx columns per call
KG = 4               # k-slices per DVE subtract / PSUM tile

_NC_CACHE = {}


def build_nc():
    nc = bacc.Bacc("TRN2", target_bir_lowering=False)
    f32 = mybir.dt.float32
    xtr_d = nc.dram_tensor("xtr", [P, N], f32, kind="ExternalInput").ap()
    xcb_d = nc.dram_tensor("xcb", [P, NBLK * C], f32, kind="ExternalInput").ap()
    idxw_d = nc.dram_tensor(
        "idxw", [P, NI * CPI], mybir.dt.int16, kind="ExternalInput"
    ).ap()
    out = nc.dram_tensor(
        "out", [N, 2 * C * K], f32, kind="ExternalOutput"
    ).ap()
    # pair j covers HBM rows [j*256, (j+1)*256); view [p, h, f] matches the
    # SBUF pair tile (h = block half, 8 KB contiguous per (p, h)).
    out_pairs = out.rearrange("(j h p) f -> j p h f", h=2, p=P)

    with TileContext(nc) as tc:
        with (
            tc.tile_pool(name="const", bufs=1) as cpool,
            tc.tile_pool(name="gat", bufs=2) as gpool,
            tc.tile_pool(name="psum", bufs=8, space="PSUM") as ppool,
            tc.tile_pool(name="outp", bufs=4) as opool,
        ):
            idx_sb = cpool.tile([P, NI * CPI], mybir.dt.int16)
            for t in range(NI):
                nc.sync.dma_start(
                    idx_sb[:, t * CPI : (t + 1) * CPI],
                    idxw_d[:, t * CPI : (t + 1) * CPI],
                )
            xtr_sb = cpool.tile([P, N], f32)
            XCH = 4
            for ch in range(XCH):
                s = ch * (N // XCH)
                e = (ch + 1) * (N // XCH)
                nc.sync.dma_start(xtr_sb[:, s:e], xtr_d[:, s:e])
            xcb_sb = cpool.tile([P, NBLK * C], f32)
            nc.sync.dma_start(xcb_sb[:], xcb_d)
            ident = cpool.tile([P, P], f32)
            make_identity(nc, ident[:])

            xtr_3d = xtr_sb[:].rearrange("p (n one) -> p n one", one=1)
            for t in range(NI):
                g = gpool.tile([P, NIDX], f32)
                nc.gpsimd.ap_gather(
                    out_ap=g[:].rearrange("p (l one) -> p l one", one=1),
                    in_ap=xtr_3d,
                    idxs_ap=idx_sb[:, t * CPI : (t + 1) * CPI],
                    channels=P,
                    num_elems=N,
                    d=1,
                    num_idxs=NIDX,
                )
                for q in range(GB):
                    nb = t * 2 * GB + 2 * q  # first block of the pair
                    ot = opool.tile([P, 2 * 2 * C * K], f32)
                    ot_v = ot[:].rearrange("p (h c2 k) -> p h c2 k", h=2, k=K)
                    cen = (
                        xcb_sb[:, nb * C : (nb + 2) * C]
                        .rearrange("p (h c) -> p h c", h=2)
                        .unsqueeze(3)
                    )
                    # second output half: center broadcast over k (ACT)
                    nc.scalar.copy(
                        ot_v[:, :, C : 2 * C, :],
                        cen.broadcast_to([P, 2, C, K]),
                    )
                    cen_kg = cen.broadcast_to([P, 2, C, KG])
                    for kg in range(K // KG):
                        ps = ppool.tile([P, KG * P], f32)
                        for kq in range(KG):
                            k = kg * KG + kq
                            nc.tensor.transpose(
                                ps[:, kq * P : (kq + 1) * P],
                                g[:, q * K * P + k * P : q * K * P + (k + 1) * P],
                                ident[:],
                            )
                        # first output half: neighbor - center (DVE, 4 k's)
                        nc.vector.tensor_sub(
                            ot_v[:, :, 0:C, kg * KG : (kg + 1) * KG],
                            ps[:].rearrange("p (kq h c) -> p h c kq", kq=KG, h=2),
                            cen_kg,
                        )
                    nc.sync.dma_start(
                        out_pairs[t * GB + q],
                        ot[:].rearrange("p (h f) -> p h f", h=2),
                    )
    nc.compile()
    return nc


def get_nc():
    if "nc" not in _NC_CACHE:
        _NC_CACHE["nc"] = build_nc()
    return _NC_CACHE["nc"]


def _prep_inputs(x: np.ndarray, idx: np.ndarray):
    """Host-side layout prep (pure data movement, no compute).

    xtr:  [B, 128, N]  x transposed, channels duplicated on both halves.
    xcb:  [B, 128, NBLK*C]  x with n = nb*128 + p -> [p, nb*C + c].
    idxw: [B, 128, NI*CPI] int16 wrapped gather lists. Per call t the
          16-wrapped list of partition band g covers blocks
          base+2q (bands 0-3) / base+2q+1 (bands 4-7), base = t*2*GB,
          in l = k*128 + p order, q-major.
    """
    x = np.ascontiguousarray(np.asarray(x, dtype=np.float32))
    xt = x.transpose(0, 2, 1)  # (B, C, N)
    xtr = np.ascontiguousarray(np.concatenate([xt, xt], axis=1))
    xcb = np.ascontiguousarray(
        x.reshape(B, NBLK, P, C).transpose(0, 2, 1, 3).reshape(B, P, NBLK * C)
    )
    idx16 = np.asarray(idx).astype(np.int16).reshape(B, NBLK, P, K)
    flat = idx16.transpose(0, 1, 3, 2).reshape(B, NBLK, K * P)  # l = k*128+p
    idxw = np.empty((B, P, NI * CPI), np.int16)
    for t in range(NI):
        base = t * 2 * GB
        a = flat[:, base : base + 2 * GB : 2, :].reshape(B, NIDX)
        b = flat[:, base + 1 : base + 2 * GB : 2, :].reshape(B, NIDX)
        wa = a.reshape(B, CPI, 16).transpose(0, 2, 1)  # (B, 16, CPI)
        wb = b.reshape(B, CPI, 16).transpose(0, 2, 1)
        idxw[:, 0:64, t * CPI : (t + 1) * CPI] = np.tile(wa, (1, 4, 1))
        idxw[:, 64:128, t * CPI : (t + 1) * CPI] = np.tile(wb, (1, 4, 1))
    return xtr, xcb, idxw


def run_on_hw(x: np.ndarray, idx: np.ndarray, **spmd_kwargs):
    """Run the bass kernel on 8 NeuronCores. Returns (out, BassKernelResults)."""
    xtr, xcb, idxw = _prep_inputs(x, idx)
    in_maps = [
        {"xtr": xtr[b], "xcb": xcb[b], "idxw": idxw[b]} for b in range(B)
    ]
    res = run_bass_kernel_spmd(
        get_nc(), in_maps, core_ids=list(range(B)), **spmd_kwargs
    )
    out = np.stack([r["out"].reshape(N, 2 * C, K) for r in res.results])
    return out, res


def kernel(x: np.ndarray, idx: np.ndarray) -> np.ndarray:
    out, _ = run_on_hw(x, idx)
    return out


# revision 6
# speedup vs baseline: 5.7532x; 5.7532x over previous
"""Trainium2 Bass kernel (v10) for LocalDynamicGraph edge-feature construction.

Per batch element b (one NeuronCore each, data-parallel over B=8):
    out[b, n, c, k] = x[b, idx[b,n,k], c] - x[b, n, c]   for c < 64
    out[b, n, c, k] = x[b, n, c - 64]                    for c >= 64

v10 = v8's SWDGE-gather pipeline, stripped to its engine-bus floor.
Trace analysis of v8 showed the SDMA engines are the bottleneck: per
128B gather descriptor ~13ns of engine-bus time (desc fetch + HBM read
+ SBUF write) plus 64 MiB of output writes at ~25 GB/s/engine; Pool
desc-gen itself is fast (~0.1us/call; its long trace slices were ring
backpressure).  So v10 removes everything else from the bus and the
critical path:

  - xbf (bf16 table, 256B-stride rows) is host-prepped and loaded
    directly (2 MB) instead of being staged x->SBUF->cast->HBM; the
    fp32 warm-phase gathers (256B descriptors) are gone - all 64
    blocks gather bf16/2048-idx/per-descriptor-packet calls.
  - centers are host-laid-out contiguously (xcb [128, 64*64] fp32,
    partition = point-in-block) so the 2 MB load is 16KB/partition
    contiguous instead of 8192 x 256B descriptors.
  - bigger SWDGE descriptor rings (128 KB scratch) so Pool never
    stalls the queues.
  - DVE computes (neighbor - center) via a strided bf16 view, ACT
    broadcast-copies the fp32 center half (bit-exact), each block is
    written back as one fully contiguous 1 MB DMA.
"""

import numpy as np

import concourse.bacc as bacc
import concourse.mybir as mybir
from concourse.tile import TileContext
from concourse.bass_utils import run_bass_kernel_spmd

# Problem constants (hardcoded per contest contract).
B = 8
N = 8192
C = 64
K = 16
P = 128              # partitions / points per output tile
NBLK = N // P        # 64 point-blocks per core
NQ = 4               # SWDGE queues (ucode max)
GS = K * P           # indices per gather call (one block = 2048)
XPAD = 128           # bf16 row padded to 128 elements = 256B stride
COLS = NBLK * GS // 16   # idxw columns = 8192

_NC_CACHE = {}


def _dma_gather_raw(gp, out_ap, in_ap, idxs_ap, num_idxs, num_idxs_reg,
                    elem_size, elem_step, queue_num, single_packet=True):
    """bass.dma_gather minus the elem_size%256B assert. The SWDGE ucode
    only needs the source stride (elem_step bytes) to be a multiple of
    256; the per-index payload is a plain descriptor length."""
    dtsize = mybir.dt.size(in_ap.dtype)
    assert in_ap.dtype == out_ap.dtype
    assert idxs_ap.dtype == mybir.dt.int16
    stride_bytes = elem_step * dtsize
    assert stride_bytes % 256 == 0
    stride_256 = stride_bytes // 256
    assert 0 < stride_256 < 256
    assert in_ap.ap[0][0] == elem_step
    assert in_ap.ap[-1][1] == elem_size
    assert out_ap.ap[-1][1] == elem_size
    assert out_ap.ap[0][1] * out_ap.ap[1][1] == num_idxs
    _in_ap = gp.lower_ap_dma(in_ap, for_custom_bir_dma=True)
    _idxs_ap = gp.lower_ap(idxs_ap)
    _out_ap = gp.lower_ap(out_ap)
    return gp.add_instruction(
        mybir.InstDMAGatherAnt(
            name=gp.bass.get_next_instruction_name(),
            ins=[
                *_in_ap,
                _idxs_ap,
                gp.lower_val_access(gp.to_reg(num_idxs_reg)),
            ],
            outs=[_out_ap],
            transpose=False,
            num_idxs=num_idxs,
            elem_size=elem_size,
            stride_bytes_256=stride_256,
            gen_mode=0,
            single_packet=single_packet,
            queue_num=queue_num,
            sbuf_tokens_per_rank=0,
            sbuf_free_dim_per_rank=0,
            sbuf_free_dim_pad_per_rank=0,
            sbuf_byte_offset=0,
        )
    )


def build_nc():
    nc = bacc.Bacc(
        "TRN2",
        target_bir_lowering=False,
        dynamic_dma_scratch_size=65536,
        num_swdge_queues=NQ,
    )
    xbf = nc.dram_tensor(
        "xbf", [N, XPAD], mybir.dt.bfloat16, kind="ExternalInput"
    ).ap()
    xcb_d = nc.dram_tensor(
        "xcb", [P, NBLK * C], mybir.dt.float32, kind="ExternalInput"
    ).ap()
    idxw_d = nc.dram_tensor(
        "idxw", [P, COLS], mybir.dt.int16, kind="ExternalInput"
    ).ap()
    out = nc.dram_tensor(
        "out", [N, 2 * C * K], mybir.dt.float32, kind="ExternalOutput"
    ).ap()
    out_blocks = out.rearrange("(nb p) f -> nb p f", p=P)

    with TileContext(nc) as tc:
        with (
            tc.tile_pool(name="const", bufs=1) as cpool,
            tc.tile_pool(name="gat", bufs=8) as gpool,
            tc.tile_pool(name="outp", bufs=6) as opool,
        ):
            # Wrapped indices, replicated across all 16-partition groups.
            # Loaded per-call-chunk so early gathers aren't gated on 2MB.
            idx_sb = cpool.tile([P, COLS], mybir.dt.int16)
            IDX_CHUNKS = 16
            ccols = COLS // IDX_CHUNKS
            for ch in range(IDX_CHUNKS):
                nc.sync.dma_start(
                    idx_sb[:, ch * ccols : (ch + 1) * ccols],
                    idxw_d[:, ch * ccols : (ch + 1) * ccols],
                )
            # Centers, host-laid-out [p, nb*C + c]; contiguous 16KB/partition.
            xcb_sb = cpool.tile([P, NBLK * C], mybir.dt.float32)
            nc.sync.dma_start(xcb_sb[:], xcb_d)

            xbf_src = xbf[:, 0:C]  # ap [(XPAD, N), (1, C)]: 256B stride, 128B payload
            nreg = nc.gpsimd.to_reg(GS)
            for nb in range(NBLK):
                gt = gpool.tile([P, K * C], mybir.dt.bfloat16)
                _dma_gather_raw(
                    nc.gpsimd,
                    out_ap=gt[:].rearrange("p (g c) -> p g c", c=C),
                    in_ap=xbf_src,
                    idxs_ap=idx_sb[:, nb * (GS // 16) : (nb + 1) * (GS // 16)],
                    num_idxs=GS,
                    num_idxs_reg=nreg,
                    elem_size=C,
                    elem_step=XPAD,
                    queue_num=nb % NQ,
                    # >64-desc concatenated packets hang the SDMA.
                    single_packet=False,
                )
                ot = opool.tile([P, 2 * C * K], mybir.dt.float32)
                neigh = (
                    gt[:].rearrange("p (r c) -> p r c", c=C).transpose([0, 2, 1])
                )  # (P, C, K) strided view of the k-major gathered rows
                centr = xcb_sb[:, nb * C : (nb + 1) * C]  # (P, C)
                centr_b = centr.unsqueeze(2).broadcast_to([P, C, K])
                dst1 = ot[:, 0 : C * K].rearrange("p (c k) -> p c k", k=K)
                dst2 = ot[:, C * K : 2 * C * K].rearrange("p (c k) -> p c k", k=K)
                nc.vector.tensor_sub(dst1, neigh, centr_b)
                nc.scalar.copy(dst2, centr_b)
                nc.sync.dma_start(out_blocks[nb], ot[:])
    nc.compile()
    return nc


def get_nc():
    if "nc" not in _NC_CACHE:
        _NC_CACHE["nc"] = build_nc()
    return _NC_CACHE["nc"]


def _prep_inputs(x: np.ndarray, idx: np.ndarray):
    """Host-side layout prep (pure layout/precision, no gather/arith).

    xbf:  (B, N, 128) bf16 - x rows padded to 256B stride.
    xcb:  (B, 128, NBLK*C) fp32 - centers, partition = point-in-block.
    idxw: (B, 128, COLS) int16 - per-block k-major (l = k*128+p) gather
          lists, 16-wrapped (l%16 -> partition row, l//16 -> column) and
          replicated across the eight 16-partition GPSIMD core groups.
    """
    x = np.ascontiguousarray(np.asarray(x, dtype=np.float32))
    bf16 = mybir.dt.np(mybir.dt.bfloat16)
    xbf = np.zeros((B, N, XPAD), dtype=bf16)
    xbf[:, :, 0:C] = x.astype(bf16)
    xcb = np.ascontiguousarray(
        x.reshape(B, NBLK, P, C).transpose(0, 2, 1, 3).reshape(B, P, NBLK * C)
    )
    idx16 = np.asarray(idx).astype(np.int16).reshape(B, NBLK, P, K)
    flat = idx16.transpose(0, 1, 3, 2).reshape(B, NBLK, K * P)  # l = k*128+p
    wrapped = (
        flat.reshape(B, NBLK, GS // 16, 16)
        .transpose(0, 3, 1, 2)
        .reshape(B, 16, COLS)
    )
    rep = np.broadcast_to(wrapped[:, None, :, :], (B, 8, 16, COLS))
    idxw = np.ascontiguousarray(rep.reshape(B, P, COLS))
    return xbf, xcb, idxw


def run_on_hw(x: np.ndarray, idx: np.ndarray, **spmd_kwargs):
    """Run the bass kernel on 8 NeuronCores. Returns (out, BassKernelResults)."""
    xbf, xcb, idxw = _prep_inputs(x, idx)
    in_maps = [
        {"xbf": xbf[b], "xcb": xcb[b], "idxw": idxw[b]} for b in range(B)
    ]
    res = run_bass_kernel_spmd(
        get_nc(), in_maps, core_ids=list(range(B)), **spmd_kwargs
    )
    out = np.stack([r["out"].reshape(N, 2 * C, K) for r in res.results])
    return out, res


def kernel(x: np.ndarray, idx: np.ndarray) -> np.ndarray:
    out, _ = run_on_hw(x, idx)
    return out
